# revision 11
# baseline (speedup 1.0000x reference)
"""Trainium2 Bass kernel v3 for gated multi-head attention with additive bias.

Reference (b=2, n=2048, dim=256, h=8, dh=32):
    q = x @ Wq;  k,v = split(x @ Wkv);  dots = q k^T / sqrt(dh) + attn_bias
    attn = softmax(dots);  out = attn @ v
    out = out * sigmoid(x @ Wg + bg);  return out @ Wout + bout

Sharding: 16 (batch, head) pairs -> 8 cores, 2 heads each (core c handles
batch c//4, heads 2*(c%4), 2*(c%4)+1).  Host ships exp(bias)^T in bf16 and
sums the per-core partial outputs.

v3 design (vs v2 @134us):
  * m-major sweeps: per head, all 16 j-tiles for i-half 0, then i-half 1.
    Each AV psum bank completes one sweep early, so gating, row-sum
    reciprocal, projection and output DMA for each i-half overlap the next
    sweep.  Tail shrinks from ~27us to ~8us.
  * Split cb DMA (wqk + x half-0 first) + native-row-group first S chunks:
    first exp at ~9us instead of ~31us.  Gate matmuls and v-projections
    are spread one-part-per-chunk through the first sweep.
  * Gates via the EXP table: host ships -Wg/-bg, ACT computes e^{-z},
    Pool adds 1, gated evac divides (DVE divide is 2x_1p like mult).
    No ACT table swaps; gate activations sit anywhere in the exp stream.
  * One-tile psum ring (3 x [128,1024]) allows [128,2048] wide exp over
    adjacent slots (amortizes the ACT fixed overhead).
  * Pool engine offload: normalize (tensor_scalar), v/vaug assembly,
    gate +1 evac, and output DMAs (SWDGE) keep DVE/SP under the ACT roof.
  * Projections, v matmuls and sums-transpose use whichever AV psum bank
    is idle, not ring slots (keeps exp pairing intact).

Toolchain: walrus accepts at most ONE semaphore wait per compute-engine
instruction; _split_multi_waits moves extras onto same-engine NOPs.
"""

import os
import sys

import numpy as np

for _p in ("/opt/trn_rl_repo", "/root/.axon_site/_ro/trn_rl_repo"):
    if os.path.isdir(_p) and _p not in sys.path:
        sys.path.insert(0, _p)

B = 2
N = 2048
DIM = 256
HEADS = 8
DH = 32
HPC = 2
NCORES = 8
P = 128
NT = N // P          # 16 j-tiles
NQ = N // 512        # 4 query chunks of 512

WIDE_EXP = True
GATE_VIA_EXP = False


def build_nc(split_waits=True):
    import concourse.bass as bass
    import concourse.mybir as mybir
    from concourse.alu_op_type import AluOpType
    from concourse.bass import ts
    from concourse.tile import TileContext

    f32 = mybir.dt.float32
    bf16 = mybir.dt.bfloat16
    Act = mybir.ActivationFunctionType

    n, dim, nt = N, DIM, NT
    nck = dim // P               # 2 contraction chunks over model dim
    cw = const_width()

    from concourse import tile_sem_assignment as _tsa

    _swdge_prev = _tsa.NUM_SWDGE_GLOBAL_SEMS

    nc = bass.Bass()

    cb = nc.declare_dram_parameter("cb", [P, cw], bf16, isOutput=False)
    biasT = nc.declare_dram_parameter("biasT", [HPC, n, n], bf16, isOutput=False)
    out_ext = nc.declare_dram_parameter("out", [HPC, n, dim], bf16, isOutput=True)

    _tsa.NUM_SWDGE_GLOBAL_SEMS = 1
    with TileContext(nc) as tc:
        with (
            tc.tile_pool(name="consts", bufs=1) as consts,
            tc.tile_pool(name="dram", bufs=4, space="DRAM") as dpool,
            tc.tile_pool(name="ring", bufs=1, space="PSUM") as ringpool,
            tc.tile_pool(name="av_ps", bufs=1, space="PSUM") as avpool,
            tc.tile_pool(name="bias", bufs=16) as bpool,
            tc.tile_pool(name="attn", bufs=12) as apool,
            tc.tile_pool(name="et", bufs=4) as etpool,
        ):
            # ---- packed constant DMA, split: [wqk | x half0] first ----
            cb_sb = consts.tile([P, cw], bf16, tag="cb", name="cb_sb")
            split_col = nck * P + 2 * 1024          # wqk + x(half0, c0/c1)
            nc.sync.dma_start(out=cb_sb[:, 0:split_col], in_=cb[:, 0:split_col])

            bt_tiles = {}

            def bias_dma(h, jc):
                t = bpool.tile([P, n], bf16, tag="bias", name=f"bt{h}_{jc}")
                bt_tiles[(h, jc)] = t
                nc.sync.dma_start(out=t, in_=biasT[h, ts(jc, P), :])

            bias_dma(0, 0)
            bias_dma(0, 1)
            nc.sync.dma_start(out=cb_sb[:, split_col:cw],
                              in_=cb[:, split_col:cw])
            for jc in range(2, nt):
                bias_dma(0, jc)

            off = 0

            def take(cols):
                nonlocal off
                ap = cb_sb[:, off : off + cols]
                off += cols
                return ap

            # per chunk c, cols = [q_h0|q_h1|k_h0|k_h1] x 32
            wqk = take(nck * P).rearrange("p (c m) -> p c m", c=nck)
            # x^T blocks: [half, c] each [P, 1024]
            xblk = take(4 * 1024).rearrange("p (hf c n) -> p hf c n",
                                            hf=2, c=nck)
            # gate weights (negated on host): per chunk c, cols = [g_h0|g_h1]
            wgg = take(nck * 2 * DH).rearrange("p (c m) -> p c m", c=nck)
            # v weights: per chunk c, cols = [v_h0|v_h1] x 32
            wvv = take(nck * 2 * DH).rearrange("p (c m) -> p c m", c=nck)
            # wout replicated on partition rows 0-31 and 64-95
            wout_cols = take(HPC * dim)
            wout_h = [wout_cols[0:DH, h * dim : (h + 1) * dim] for h in range(HPC)]
            wout_h_odd = [wout_cols[64 : 64 + DH, h * dim : (h + 1) * dim]
                          for h in range(HPC)]
            bg_cols = take(HPC)          # holds -bg when GATE_VIA_EXP
            bg_h = [bg_cols[0:DH, h : h + 1] for h in range(HPC)]
            eye_cols = take(16)
            eye16 = eye_cols[0:16, :]
            assert off == cw

            # ---- persistent SBUF ----
            qT_h = [consts.tile([P, n], bf16, tag=f"qT{h}", name=f"qT{h}")
                    for h in range(HPC)]
            kT_h = [consts.tile([P, n], bf16, tag=f"kT{h}", name=f"kT{h}")
                    for h in range(HPC)]
            # gates: holds 1 + e^{-z} (divide denominator); ones row at DH
            gT_h = [consts.tile([DH + 1, n], bf16, tag=f"gT{h}", name=f"gT{h}")
                    for h in range(HPC)]
            gT_odd = [consts.tile([97, n], bf16, tag=f"gTo{h}", name=f"gTo{h}")
                      for h in range(HPC)]
            vtmp = consts.tile([P, nt, 2 * DH], bf16, tag="vtmp", name="vtmp")
            # per head: [v_h | ones] (33 cols) per j-tile
            vaug = consts.tile([P, nt, HPC, DH + 1], bf16, tag="vaug", name="vaug")
            gatedT_h = [consts.tile([DH + 1, n], bf16, tag=f"gatedT{h}",
                                    name=f"gatedT{h}") for h in range(HPC)]
            godT_h = [consts.tile([97, n], bf16, tag=f"godT{h}", name=f"godT{h}")
                      for h in range(HPC)]
            recip_h = [consts.tile([P, nt], f32, tag=f"recip{h}",
                                   name=f"recip{h}") for h in range(HPC)]
            o_all = [consts.tile([P, nt, dim], bf16, tag=f"o{h}", name=f"o{h}")
                     for h in range(HPC)]
            sums16 = [[consts.tile([8, P], bf16, tag=f"s16_{h}_{hf}",
                                   name=f"s16_{h}_{hf}") for hf in range(2)]
                      for h in range(HPC)]

            zrow = consts.tile([1, 512], bf16, tag="zrow", name="zrow")
            nc.gpsimd.memset(zrow, 0.0)

            # ---- PSUM: one 3-slot ring tile (6 banks) + av (2 banks) ----
            ring = ringpool.tile([P, 3 * 1024], f32, tag="ring", name="ring")
            av = avpool.tile([P, 1024], f32, tag="av", name="av")
            slot_ctr = [0]

            def slot():
                s = slot_ctr[0] % 3
                slot_ctr[0] += 1
                return s

            # =========== prologue helpers ===========
            def emit_qk(i, half):
                # i: 0 = q (heads 0/1), 1 = k (heads 0/1); half = i-range.
                sA, sB = slot(), slot()
                qkA = ring[0:DH, sA * 1024 : sA * 1024 + 1024]
                qkB = ring[DH : 2 * DH, sB * 1024 : sB * 1024 + 1024]
                dsts = (qT_h, kT_h)[i]
                for c in range(nck):
                    for s in range(2):
                        for e in range(2):
                            out = (qkA[:, ts(s, 512)] if e == 0
                                   else qkB[:, ts(s, 512)])
                            nc.tensor.matmul(
                                out,
                                wqk[:, c, (2 * i + e) * DH :
                                    (2 * i + e + 1) * DH],
                                xblk[:, half, c, ts(s, 512)],
                                start=(c == 0),
                                stop=(c == nck - 1),
                                skip_group_check=True,
                            )
                cols = slice(half * 1024, (half + 1) * 1024)
                nc.vector.tensor_copy(dsts[0][0:DH, cols], qkA)
                nc.vector.tensor_copy(dsts[1][DH : 2 * DH, cols], qkB)

            def replicate_qk(half):
                cols = slice(half * 1024, (half + 1) * 1024)
                for h in range(HPC):
                    for tile in (qT_h[h], kT_h[h]):
                        if h == 0:
                            nc.sync.dma_start(out=tile[DH : 2 * DH, cols],
                                              in_=tile[0:DH, cols])
                        else:
                            nc.sync.dma_start(out=tile[0:DH, cols],
                                              in_=tile[DH : 2 * DH, cols])
                for h in range(HPC):
                    for tile in (qT_h[h], kT_h[h]):
                        nc.sync.dma_start(out=tile[2 * DH : 4 * DH, cols],
                                          in_=tile[0 : 2 * DH, cols])

            # gates: one part = (head, s-pair); 4 matmuls + 2 activations
            def emit_gates_part(h, spair):
                sg = slot()
                for sub in range(2):
                    s = 2 * spair + sub
                    gps = ring[0:DH,
                               sg * 1024 + sub * 512 :
                               sg * 1024 + (sub + 1) * 512]
                    for c in range(nck):
                        nc.tensor.matmul(
                            gps,
                            wgg[:, c, h * DH : (h + 1) * DH],
                            xblk[:, s // 2, c, ts(s % 2, 512)],
                            start=(c == 0),
                            stop=(c == nck - 1),
                            skip_group_check=True,
                        )
                    if GATE_VIA_EXP:
                        # e^{-z}: gps = x@(-Wg), bias = -bg
                        et_g = etpool.tile([DH, 512], bf16, tag="et",
                                           name="etg", padded_shape=[P, 2048])
                        nc.scalar.activation(
                            out=et_g, in_=gps, func=Act.Exp,
                            scale=1.0, bias=bg_h[h],
                        )
                        # g = 1 + e^{-z} on Pool
                        nc.gpsimd.tensor_scalar_add(
                            gT_h[h][0:DH, ts(s, 512)], et_g, 1.0
                        )
                    else:
                        nc.scalar.activation(
                            out=gT_h[h][0:DH, ts(s, 512)],
                            in_=gps, func=Act.Sigmoid,
                            scale=1.0, bias=bg_h[h],
                        )

            def emit_gates_tail():
                for h in range(HPC):
                    nc.gpsimd.memset(gT_h[h][DH : DH + 1, :], 1.0)
                    nc.sync.dma_start(out=gT_odd[h][64:97, :],
                                      in_=gT_h[h][0:33, :])

            # v projections into the (still unopened) av banks
            def emit_v(t):
                vcol = (0 if t < 8 else 512) + (t % 8) * 64
                vps = av[:, vcol : vcol + 64]
                for c in range(nck):
                    nc.tensor.matmul(
                        vps,
                        xblk[:, t // 8, c, (t % 8) * P : (t % 8) * P + P],
                        wvv[:, c, :],
                        start=(c == 0),
                        stop=(c == nck - 1),
                        skip_group_check=True,
                    )
                nc.vector.tensor_copy(vtmp[:, t, :], vps)

            def emit_vaug():
                nc.gpsimd.memset(vaug[:, :, :, DH : DH + 1], 1.0)
                nc.gpsimd.tensor_copy(
                    vaug[:, :, :, 0:DH],
                    vtmp.rearrange("p t (h d) -> p t h d", h=HPC),
                )

            # =========== evacuation helpers ===========
            def emit_gated(h, q):
                qcol = 512 * (q // 2)
                if q % 2 == 0:
                    dst = gatedT_h[h][:, ts(q, 512)]
                    src = av[0 : DH + 1, qcol : qcol + 512]
                    g = gT_h[h][:, ts(q, 512)]
                else:
                    dst = godT_h[h][64:97, ts(q, 512)]
                    src = av[64 : 64 + DH + 1, qcol : qcol + 512]
                    g = gT_odd[h][64:97, ts(q, 512)]
                if GATE_VIA_EXP:
                    nc.vector.tensor_tensor(dst, src, g, AluOpType.divide)
                else:
                    nc.vector.tensor_mul(dst, src, g)

            scr_half = {}

            def emit_strip(h, q):
                key = (h, q // 2)
                if key not in scr_half:
                    scr_half[key] = dpool.tile([1024], bf16, tag="scr",
                                               name=f"scr{h}_{q//2}")
                scr_ = scr_half[key]
                if q % 2 == 0:
                    strip = gatedT_h[h][DH : DH + 1, ts(q, 512)]
                else:
                    strip = godT_h[h][96:97, ts(q, 512)]
                nc.sync.dma_start(out=scr_[ts(q % 2, 512)], in_=strip)

            def emit_recip(h, hf, free_bank):
                scr_ = scr_half[(h, hf)]
                s16 = sums16[h][hf]
                nc.sync.dma_start(
                    out=s16, in_=scr_[:].rearrange("(t p) -> t p", p=P)
                )
                spt = av[:, free_bank * 512 + 256 : free_bank * 512 + 264]
                nc.tensor.matmul(spt, s16, eye16[0:8, 0:8],
                                 start=True, stop=True, skip_group_check=True)
                nc.vector.reciprocal(
                    recip_h[h][:, hf * 8 : hf * 8 + 8], spt
                )

            proj_flip = [0]

            def emit_proj(h, t, free_bank):
                pq = proj_flip[0] % 2
                proj_flip[0] += 1
                pp = av[:, free_bank * 512 + pq * 256 :
                        free_bank * 512 + pq * 256 + 256]
                if (t // 4) % 2 == 0:
                    lhsT = gatedT_h[h][0:DH, ts(t, P)]
                    rhs = wout_h[h]
                else:
                    lhsT = godT_h[h][64 : 64 + DH, ts(t, P)]
                    rhs = wout_h_odd[h]
                nc.tensor.matmul(pp, lhsT, rhs, start=True, stop=True,
                                 skip_group_check=True)
                nc.vector.tensor_scalar_mul(
                    o_all[h][:, t, :], pp, recip_h[h][:, t : t + 1]
                )
                nc.gpsimd.dma_start(
                    out=out_ext[h, t * P : (t + 1) * P, :],
                    in_=o_all[h][:, t, :],
                )

            # =========== main sweeps ===========
            PROJ_A = [0, 4, 1, 5, 2, 6, 3, 7]
            PROJ_B = [8, 12, 9, 13, 10, 14, 11, 15]

            def drain_bank(h_e, m_e):
                """Emit any still-pending AV chunks of bank (h_e, m_e); they
                are always the oldest pend entries (FIFO)."""
                while pend and pend[0][0] == h_e and pend[0][2] == m_e:
                    hh, jj, mm, attn_ = pend.pop(0)
                    emit_av(hh, jj, mm, attn_)

            def injected(h, m, jc):
                """Deferred work at this (h, m, jc) point.  A bank's last AV
                must be emitted before its evacuation (drain_bank)."""
                if m == 1:
                    if jc == 3:
                        drain_bank(h, 0)
                        emit_gated(h, 0)
                        emit_gated(h, 1)
                    elif jc == 4:
                        emit_strip(h, 0)
                        emit_strip(h, 1)
                    elif jc == 5:
                        emit_recip(h, 0, free_bank=0)
                    elif 7 <= jc <= 14:
                        emit_proj(h, PROJ_A[jc - 7], free_bank=0)
                if m == 0 and h >= 1:
                    hp = h - 1
                    if jc == 3:
                        drain_bank(hp, 1)
                        emit_gated(hp, 2)
                        emit_gated(hp, 3)
                    elif jc == 4:
                        emit_strip(hp, 2)
                        emit_strip(hp, 3)
                    elif jc == 5:
                        emit_recip(hp, 1, free_bank=1)
                    elif 7 <= jc <= 14:
                        emit_proj(hp, PROJ_B[jc - 7], free_bank=1)

            cc = 0
            pend = []
            opened = {}

            def open_bank(m_):
                nc.tensor.matmul(
                    av[:, 512 * m_ : 512 * m_ + 512],
                    zrow[0:1, 0:P],
                    zrow[0:1, 0:512],
                    start=True,
                    stop=False,
                    skip_group_check=True,
                )

            def emit_av(hh, jc_, m_, attn_):
                if not opened.get((hh, m_)):
                    open_bank(m_)
                    opened[(hh, m_)] = True
                for e in range(2):
                    nc.tensor.matmul(
                        av[64 * e : 64 * e + DH + 1,
                           512 * m_ : 512 * m_ + 512],
                        vaug[:, jc_, hh, :],
                        attn_[:, ts(e, 512)],
                        start=False,
                        stop=(jc_ == nt - 1 and e == 1),
                        skip_group_check=True,
                    )

            def emit_chunk(h, m, jc, first_sweep):
                nonlocal cc
                sl = slot()
                sps = ring[:, sl * 1024 : (sl + 1) * 1024]
                for s in range(2):
                    g = (2 * cc + s) % 4
                    if first_sweep and cc == 0:
                        g = 0            # native rows before replication
                    elif first_sweep and cc == 1:
                        g = s            # row 32-63 after hop A only
                    nc.tensor.matmul(
                        sps[:, ts(s, 512)],
                        kT_h[h][32 * g : 32 * (g + 1), ts(jc, P)],
                        qT_h[h][32 * g : 32 * (g + 1),
                                m * 1024 + s * 512 : m * 1024 + (s + 1) * 512],
                        start=True,
                        stop=True,
                        tile_position=(32 * g, 0),
                        skip_group_check=True,
                    )
                cc += 1
                return sl

            def emit_exp_mul(h, m, jc, sl, wide_partner=None):
                """exp+mul for chunk; wide_partner=(h2,m2,jc2,sl2) pairs
                two ring slots into one [128,2048] ACT read."""
                if wide_partner is not None:
                    h2, m2, jc2, _sl2 = wide_partner
                    et = etpool.tile([P, 2048], bf16, tag="et", name="et",
                                     padded_shape=[P, 2048])
                    nc.scalar.activation(
                        out=et, in_=ring[:, sl * 1024 : sl * 1024 + 2048],
                        func=Act.Exp,
                    )
                    for idx, (hh, mm, jj) in enumerate(((h, m, jc),
                                                        (h2, m2, jc2))):
                        attn = apool.tile([P, 1024], bf16, tag="attn",
                                          name="attn")
                        nc.vector.tensor_mul(
                            attn, et[:, idx * 1024 : (idx + 1) * 1024],
                            bt_tiles[(hh, jj)][:, mm * 1024 : (mm + 1) * 1024],
                        )
                        pend.append((hh, jj, mm, attn))
                else:
                    et = etpool.tile([P, 1024], bf16, tag="et", name="et",
                                     padded_shape=[P, 2048])
                    nc.scalar.activation(
                        out=et, in_=ring[:, sl * 1024 : (sl + 1) * 1024],
                        func=Act.Exp,
                    )
                    attn = apool.tile([P, 1024], bf16, tag="attn", name="attn")
                    nc.vector.tensor_mul(
                        attn, et,
                        bt_tiles[(h, jc)][:, m * 1024 : (m + 1) * 1024],
                    )
                    pend.append((h, jc, m, attn))

            def drain_pend(lag):
                while len(pend) > lag:
                    hh, jj, mm, attn_ = pend.pop(0)
                    emit_av(hh, jj, mm, attn_)

            # ---------- emission ----------
            emit_qk(0, 0)          # q half0
            emit_qk(1, 0)          # k half0
            replicate_qk(0)
            emit_qk(0, 1)          # q half1
            emit_qk(1, 1)          # k half1
            replicate_qk(1)
            for gh in range(HPC):
                for gsp in range(2):
                    emit_gates_part(gh, gsp)
            emit_gates_tail()

            sweep_no = 0
            for h in range(HPC):
                if h == 1:
                    for jc in range(nt):
                        bias_dma(1, jc)
                for m in range(2):
                    opened[(h, m)] = False
                    first_sweep = (sweep_no == 0)
                    held = None   # (h, m, jc, slot) awaiting wide partner
                    for jc in range(nt):
                        injected(h, m, jc)
                        sl = emit_chunk(h, m, jc, first_sweep)
                        if (WIDE_EXP and held is not None
                                and held[3] + 1 == sl and held[3] != 2):
                            emit_exp_mul(held[0], held[1], held[2], held[3],
                                         wide_partner=(h, m, jc, sl))
                            held = None
                        elif held is not None:
                            emit_exp_mul(held[0], held[1], held[2], held[3])
                            held = (h, m, jc, sl)
                        elif (WIDE_EXP and sl != 2
                              and not (first_sweep and jc < 6)):
                            held = (h, m, jc, sl)
                        else:
                            emit_exp_mul(h, m, jc, sl)
                        # prologue injections during the very first sweep
                        if first_sweep:
                            if 1 <= jc <= 4:
                                for t in range(4 * (jc - 1), 4 * jc):
                                    emit_v(t)
                                if jc == 4:
                                    emit_vaug()
                        if first_sweep:
                            lag = 6 if jc < 7 else max(3, 6 - (jc - 6))
                        else:
                            lag = 3
                        drain_pend(lag)
                    if held is not None:
                        emit_exp_mul(held[0], held[1], held[2], held[3])
                        held = None
                    sweep_no += 1
            drain_pend(0)

            # =========== tail: h1 bank1 evacuation ===========
            emit_gated(1, 2)
            emit_gated(1, 3)
            emit_strip(1, 2)
            emit_strip(1, 3)
            emit_recip(1, 1, free_bank=1)
            for t in PROJ_B:
                emit_proj(1, t, free_bank=1)

    _tsa.NUM_SWDGE_GLOBAL_SEMS = _swdge_prev
    if split_waits:
        _split_multi_waits(nc)
    return nc


def _split_multi_waits(nc):
    """walrus accepts at most ONE semaphore wait per engine instruction;
    extra waits ride same-engine NOPs inserted just before (queues execute
    in order)."""
    import concourse.mybir as mybir

    n = 0
    for f in nc.m.functions:
        for blk in f.blocks:
            out = []
            changed = False
            for inst in blk.instructions:
                si = getattr(inst, "sync_info", None)
                ws = list(si.on_wait) if si and si.on_wait else []
                if len(ws) > 1:
                    for w in ws[:-1]:
                        nop = mybir.InstNoOp(
                            name=f"I-waitsplit-{n}",
                            engine=inst.engine,
                            sync_info=mybir.SyncInfo(on_wait=[w], on_update=[]),
                        )
                        out.append(nop)
                        n += 1
                    si.on_wait = [ws[-1]]
                    inst.sync_info = si
                    changed = True
                out.append(inst)
            if changed:
                blk.instructions = out


def check_mm_waits(nc):
    bad = []
    for f in nc.m.functions:
        for blk in f.blocks:
            for inst in blk.instructions:
                if type(inst).__name__ in ("InstDMACopy", "InstDrain"):
                    continue
                si = getattr(inst, "sync_info", None)
                ws = list(si.on_wait) if si and si.on_wait else []
                if len(ws) > 1:
                    bad.append(
                        (inst.name, type(inst).__name__,
                         [(w.ant_name, w.wait_value) for w in ws])
                    )
    return bad


def const_width():
    nck = DIM // P
    return (nck * P + nck * N + nck * 2 * DH + nck * 2 * DH + HPC * DIM
            + HPC + 16)


def pack_consts(xT, wq2, wk2, wv2, wg2, bg2, wout2):
    """xT [dim, n]; wq2/wk2/wv2/wg2 [dim, 2*DH] (head-major cols);
    bg2 [2*DH]; wout2 [2*DH, dim].  wg2/bg2 are pre-negated by caller."""
    nck = DIM // P
    cw = const_width()
    cbuf = np.zeros((P, cw), np.float32)
    off = 0

    def put(block, cols):
        nonlocal off
        cbuf[: block.shape[0], off : off + cols] = block
        off += cols

    wqk = np.concatenate([wq2, wk2], axis=1)  # [dim, 128]
    put(wqk.reshape(nck, P, P).transpose(1, 0, 2).reshape(P, nck * P), nck * P)
    # x blocks: (half, c) each [P, 1024]
    xb = np.zeros((P, 4 * 1024), np.float32)
    for hf in range(2):
        for c in range(nck):
            xb[:, (2 * hf + c) * 1024 : (2 * hf + c + 1) * 1024] = \
                xT[c * P : (c + 1) * P, hf * 1024 : (hf + 1) * 1024]
    put(xb, 4 * 1024)
    for w in (wg2, wv2):
        put(w.reshape(nck, P, 2 * DH).transpose(1, 0, 2).reshape(P, nck * 2 * DH),
            nck * 2 * DH)
    wout_cols = np.zeros((64 + DH, HPC * DIM), np.float32)
    for h in range(HPC):
        wout_cols[0:DH, h * DIM : (h + 1) * DIM] = wout2[h * DH : (h + 1) * DH, :]
        wout_cols[64 : 64 + DH, h * DIM : (h + 1) * DIM] = \
            wout2[h * DH : (h + 1) * DH, :]
    put(wout_cols, HPC * DIM)
    bg_cols = np.zeros((DH, HPC), np.float32)
    for h in range(HPC):
        bg_cols[:, h] = bg2[h * DH : (h + 1) * DH]
    put(bg_cols, HPC)
    put(np.eye(16, dtype=np.float32), 16)
    assert off == cw
    return cbuf


def shard_inputs(x, attn_bias, Wq, Wkv, Wg, bg, Wout):
    import ml_dtypes

    scale = DH ** -0.5
    gsign = np.float32(-1.0 if GATE_VIA_EXP else 1.0)
    in_maps = []
    for c in range(NCORES):
        b = c // 4
        hp = c % 4
        hs = slice(2 * hp * DH, (2 * hp + 2) * DH)
        cbuf = pack_consts(
            np.ascontiguousarray(x[b].T),
            Wq[:, hs] * np.float32(scale),
            Wkv[:, :DIM][:, hs],
            Wkv[:, DIM:][:, hs],
            Wg[:, hs] * gsign,
            (bg[hs] * gsign).astype(np.float32),
            Wout[hs, :],
        )
        in_maps.append(
            {
                "cb": cbuf.astype(ml_dtypes.bfloat16),
                "biasT": np.exp(
                    attn_bias[b, 2 * hp : 2 * hp + 2].transpose(0, 2, 1)
                ).astype(ml_dtypes.bfloat16),
            }
        )
    return in_maps


def gather_outputs(outs, bout):
    parts = [np.asarray(o, np.float32).sum(axis=0) for o in outs]
    out0 = parts[0] + parts[1] + parts[2] + parts[3]
    out1 = parts[4] + parts[5] + parts[6] + parts[7]
    return (np.stack([out0, out1]) + bout).astype(np.float32)


def _numpy_fallback(x, mask, attn_bias, Wq, Wkv, Wg, bg, Wout, bout):
    b, n, dim = x.shape
    h, dh = HEADS, DH
    scale = dh ** -0.5
    q = (x @ Wq).reshape(b, n, h, dh).transpose(0, 2, 1, 3)
    kv = x @ Wkv
    k = kv[..., : h * dh].reshape(b, n, h, dh).transpose(0, 2, 1, 3)
    v = kv[..., h * dh :].reshape(b, n, h, dh).transpose(0, 2, 1, 3)
    dots = np.einsum("bhid,bhjd->bhij", q * scale, k) + attn_bias
    pair = mask[:, None, :, None] & mask[:, None, None, :]
    dots = np.where(pair, dots, -np.finfo(dots.dtype).max)
    dots -= dots.max(axis=-1, keepdims=True)
    attn = np.exp(dots)
    attn /= attn.sum(axis=-1, keepdims=True)
    out = np.einsum("bhij,bhjd->bhid", attn, v)
    out = out.transpose(0, 2, 1, 3).reshape(b, n, h * dh)
    gates = 1.0 / (1.0 + np.exp(-(x @ Wg + bg)))
    return ((out * gates) @ Wout + bout).astype(np.float32)


_NC_CACHE = {}


def _get_nc():
    if "nc" not in _NC_CACHE:
        _NC_CACHE["nc"] = build_nc()
    return _NC_CACHE["nc"]


def run_on_device(in_maps, **kwargs):
    from concourse.bass_utils import run_bass_kernel_spmd

    nc = _get_nc()
    return run_bass_kernel_spmd(nc, in_maps, core_ids=list(range(NCORES)), **kwargs)


def kernel(x, mask, attn_bias, Wq, Wkv, Wg, bg, Wout, bout):
    x = np.asarray(x, np.float32)
    mask = np.asarray(mask)
    attn_bias = np.asarray(attn_bias, np.float32)
    Wq = np.asarray(Wq, np.float32)
    Wkv = np.asarray(Wkv, np.float32)
    Wg = np.asarray(Wg, np.float32)
    bg = np.asarray(bg, np.float32)
    Wout = np.asarray(Wout, np.float32)
    bout = np.asarray(bout, np.float32)

    if not mask.all():
        return _numpy_fallback(x, mask, attn_bias, Wq, Wkv, Wg, bg, Wout, bout)

    in_maps = shard_inputs(x, attn_bias, Wq, Wkv, Wg, bg, Wout)
    res = run_on_device(in_maps)
    outs = [res.results[i]["out"] for i in range(NCORES)]
    return gather_outputs(outs, bout)


if __name__ == "__main__":
    nc = build_nc()
    bad = check_mm_waits(nc)
    print("multi-wait engine instructions:", len(bad))
    for b_ in bad[:30]:
        print("  ", b_)
